# revision 46
# baseline (speedup 1.0000x reference)
"""BitNet-style attention (B=2, T=2048, D=1024, 16 heads, RoPE, causal) on
8 TRN2 NeuronCores.

Head-parallel attention (2 heads/core) with a token-parallel wo:
  - host pre-quantizes x (int-valued bf16, transposed) and the ternary
    weights; per-token dequant scales (isx) are folded into the RoPE
    tables (q,k), the exp bias (A' = A*isx_k), and an augmented V column
    (1/isx_k = sx) that yields the softmax denominator for free.
  - scores: 2 heads row-packed on the PE (K=64 each); causal blocks get
    a restricted moving dim; the 128-wide diagonal is masked post-exp.
  - batch-1 projections are interleaved into batch-0's (exp-bound)
    attention stream through a shared PSUM pool set.
  - one AllToAll per batch reshards the renormalized f32 attention out
    by token owner (128 dims + a partial-absmax row per core); owners
    compute the int8 scale, quantize, and their 512-token slice of wo
    locally -- no AllReduce / AllGather on the critical path.
"""

import math
from contextlib import ExitStack

import ml_dtypes
import numpy as np

import concourse.bass as bass
import concourse.bacc as bacc_mod
import concourse.bass_isa as bass_isa
import concourse.mybir as mybir
import concourse.tile as tile
from concourse.bass_utils import run_bass_kernel_spmd

F32 = mybir.dt.float32
F32R = mybir.dt.float32r
BF16 = mybir.dt.bfloat16
OP = mybir.AluOpType
ACT = mybir.ActivationFunctionType

B, T, D = 2, 2048, 1024
NT = B * T              # 4096 tokens
NH, HD = 16, 64
N_CORES = 8
HPC = NH // N_CORES     # heads/core = 2
DPC = HPC * HD          # dims/core = 128
RC = 12582912.0         # 1.5*2^23 round-to-nearest-even constant

TB = 512                # token block (matmul N)
NTB = NT // TB          # 8
NTT = NT // 128         # 32 token tiles
QB = 512                # q block
NQB = T // QB           # 4 per batch
NKT = T // 128          # 16 k tiles per batch
VW = 130                # vaug group width: [v_h0(64) | sx | v_h1(64) | sx]
OT = T // N_CORES       # tokens owned per core per batch = 256
PR = 129                # a2a part rows: 128 dims + 1 partial-max row
ISQ = 1.0 / math.sqrt(HD)


def _quant_w(w):
    O, I = w.shape
    wg = w.reshape(O, I // 128, 128)
    ws = np.abs(wg).mean(-1, keepdims=True) + 1e-5
    wq = np.clip(np.round(wg / ws), -1.0, 1.0) * ws
    return wq.reshape(O, I).astype(np.float32)


def build_nc():
    nc = bacc_mod.Bacc(num_devices=N_CORES)
    io = {}

    def inp(name, shape, dt=F32):
        io[name] = nc.dram_tensor(name, shape, dt, kind="ExternalInput")

    inp("xiT", [D, NT], BF16)        # quantized x, transposed (int-valued)
    inp("wall", [D, 3 * DPC + D], BF16)  # [wq|wk|wv slices | full woT]
    inp("cmx", [128, NT], BF16)      # cos table * isx
    inp("smx", [128, NT], BF16)      # sin table * isx
    inp("lnisx", [128, NTT], F32)    # ln(isx) laid out [token%128, tile]
    inp("sxp", [128, NTT], BF16)     # sx laid out [token%128, tile]
    inp("umask", [128, 128], BF16)   # tri mask (1 if q>=k)
    inp("pswapT", [128, 128], BF16)  # RoPE pair swap
    inp("identb", [128, 128], BF16)
    inp("sel16", [16, 16 * 64], F32R)   # one-hot row selectors (renorm bcast)
    inp("ones1", [1, 128], F32R)
    out = nc.dram_tensor("out", [D, 2 * OT], F32, kind="ExternalOutput")

    r32 = lambda ap: ap.bitcast(F32R)
    RG = [list(range(N_CORES))]

    with nc.allow_low_precision(reason="bf16 matmul pipeline on int-exact activations"), \
         tile.TileContext(nc) as tc, ExitStack() as top:
        cpool = top.enter_context(tc.tile_pool(name="const", bufs=1))
        dpool = top.enter_context(tc.tile_pool(name="dram", bufs=1, space="DRAM"))

        # ---------------- persistent tiles ----------------
        w_sb = [cpool.tile([128, 3 * DPC + D], BF16, name=f"w{i}", tag=f"w{i}")
                for i in range(8)]
        xi_sb = [cpool.tile([128, NT], BF16, name=f"xi{i}", tag=f"xi{i}")
                 for i in range(8)]
        lnisx = cpool.tile([128, NTT], F32, name="lnisx", tag="lnisx")
        sxp = cpool.tile([128, NTT], BF16, name="sxp", tag="sxp")
        umask = cpool.tile([128, 128], BF16, name="umask", tag="umask")
        pswapT = cpool.tile([128, 128], BF16, name="pswapT", tag="pswapT")
        identb = cpool.tile([128, 128], BF16, name="identb", tag="identb")
        sel16 = cpool.tile([16, 16 * 64], F32R, name="sel16", tag="sel16")
        ones1 = cpool.tile([1, 128], F32R, name="ones1", tag="ones1")

        qR = cpool.tile([128, NT], BF16, name="qR", tag="qR")
        kR = cpool.tile([128, NT], BF16, name="kR", tag="kR")
        vaug = cpool.tile([128, NTT * VW], BF16, name="vaug", tag="vaug")
        # unnormalized attention out + denominator row (row 64);
        # head0 cols [0:NT), head1 cols [NT:2NT)
        outU = cpool.tile([65, 2 * NT], F32, name="outU", tag="outU")
        rinv = cpool.tile([16, QB], F32, name="rinv", tag="rinv")

        # DRAM scratch: AllToAll payload = 8 parts x (128 dim rows + 1
        # partial-max row) x 256 owned tokens, f32
        a2a_in = [dpool.tile([8 * PR, OT], F32, name=f"a2a_in{c}",
                             tag=f"a2a_in{c}") for c in range(B)]
        a2a_out = [dpool.tile([8 * PR, OT], F32, name=f"a2a_out{c}",
                              tag=f"a2a_out{c}") for c in range(B)]

        # input DMAs, all on the sync ring, in consumption order
        nc.sync.dma_start(identb[:], io["identb"][:])
        for nm, t in (("lnisx", lnisx), ("sxp", sxp), ("umask", umask),
                      ("pswapT", pswapT), ("sel16", sel16), ("ones1", ones1)):
            nc.sync.dma_start(t[:], io[nm][:])
        for i in range(8):
            nc.sync.dma_start(w_sb[i][:, 0:3 * DPC],
                              io["wall"][i * 128:(i + 1) * 128, 0:3 * DPC])
        # staged rows are contracted against one-hot selectors before every
        # row is written; zero-init so 0*garbage can't produce NaN
        nc.vector.memset(rinv[:], 0.0)
        # x in 256 KB slices, token-pair-major
        for tp in range(4):
            sl = slice(tp * 1024, (tp + 1) * 1024)
            for i in range(8):
                nc.sync.dma_start(xi_sb[i][:, sl],
                                  io["xiT"][i * 128:(i + 1) * 128, sl])
        # wo weights are needed last
        for i in range(8):
            nc.sync.dma_start(w_sb[i][:, 3 * DPC:],
                              io["wall"][i * 128:(i + 1) * 128, 3 * DPC:])

        # PE warm-up: ~13 us of dependency-free matmuls so the HAM clock
        # gate opens while the input DMAs stream in
        with tc.tile_pool(name="pwm", bufs=1, space="PSUM") as pwm:
            warm = pwm.tile([128, 128], F32, name="warm", tag="warm")
            for _ in range(120):
                nc.tensor.matmul(warm[:], identb[:], identb[:],
                                 start=True, stop=True)

        # prefill vaug sx columns (cols 64 and 129 of each 130-wide group)
        for kt in range(NTT):
            nc.vector.tensor_copy(vaug[:, kt * VW + 64:kt * VW + 65],
                                  sxp[:, kt:kt + 1])
            nc.vector.tensor_copy(vaug[:, kt * VW + 129:kt * VW + 130],
                                  sxp[:, kt:kt + 1])

        # One shared PSUM pool set for both phases so their emission can be
        # interleaved: psS 2x[128,1024] (4 banks) + psA0/psA1 (2) + bb
        # 2x[128,512] (2) = 8 banks
        with tc.tile_pool(name="pps", bufs=2, space="PSUM") as pps, \
             tc.tile_pool(name="ppa", bufs=1, space="PSUM") as ppa, \
             tc.tile_pool(name="ppb", bufs=2, space="PSUM") as ppb, \
             tc.tile_pool(name="pa", bufs=2) as pa, \
             tc.tile_pool(name="pb", bufs=2) as pb, \
             tc.tile_pool(name="pbA", bufs=3) as pbA, \
             tc.tile_pool(name="pbq", bufs=1) as pbq:

            def proj_pair(tp):
                """qkv + RoPE + V transpose for token blocks 2tp, 2tp+1."""
                tbs = (2 * tp, 2 * tp + 1)
                sls = [slice(tb * TB, (tb + 1) * TB) for tb in tbs]
                cmb = pa.tile([128, 2 * TB], BF16, name="cmb", tag="cmb")
                nc.sync.dma_start(cmb[:], io["cmx"][:, tp * 1024:(tp + 1) * 1024])
                smb = pa.tile([128, 2 * TB], BF16, name="smb", tag="smb")
                nc.sync.dma_start(smb[:], io["smx"][:, tp * 1024:(tp + 1) * 1024])
                blk = {}
                for pi, pname in enumerate(("q", "k", "v")):
                    pp2 = pps.tile([128, 2 * TB], F32, name="pp2", tag="psS")
                    for i in range(8):
                        for u in range(2):
                            nc.tensor.matmul(pp2[:, u * TB:(u + 1) * TB],
                                             w_sb[i][:, pi * 128:(pi + 1) * 128],
                                             xi_sb[i][:, sls[u]],
                                             start=(i == 0), stop=(i == 7))
                    for u in range(2):
                        t = pa.tile([128, TB], BF16, name=f"t_{pname}{u}",
                                    tag=f"t_{pname}{u}")
                        if pname == "v":
                            nc.vector.tensor_copy(t[:], pp2[:, u * TB:(u + 1) * TB])
                        else:
                            nc.scalar.copy(t[:], pp2[:, u * TB:(u + 1) * TB])
                        blk[(pname, u)] = t
                for u in range(2):
                    sl = sls[u]
                    csl = slice(u * TB, (u + 1) * TB)
                    for pname, dstR in (("q", qR), ("k", kR)):
                        src = blk[(pname, u)]
                        swp = ppb.tile([128, TB], F32, name="swp", tag="bb")
                        nc.tensor.matmul(swp[:], pswapT[:], src[:],
                                         start=True, stop=True)
                        t1 = pa.tile([128, TB], BF16, name="t1", tag="t1")
                        nc.vector.tensor_tensor(t1[:], src[:], cmb[:, csl], OP.mult)
                        t2 = pa.tile([128, TB], BF16, name="t2", tag="t2")
                        nc.vector.tensor_tensor(t2[:], swp[:], smb[:, csl], OP.mult)
                        nc.vector.tensor_tensor(dstR[:, sl], t1[:], t2[:], OP.add)
                    for j in range(TB // 128):
                        kt = tbs[u] * 4 + j
                        vtp = ppb.tile([128, 128], BF16, name="vtp", tag="bb")
                        nc.tensor.transpose(vtp[:],
                                            blk[("v", u)][:, j * 128:(j + 1) * 128],
                                            identb[:])
                        nc.vector.tensor_copy(vaug[:, kt * VW:kt * VW + 64],
                                              vtp[:, 0:64])
                        nc.vector.tensor_copy(vaug[:, kt * VW + 65:kt * VW + 129],
                                              vtp[:, 64:128])

            def attention_block(bi, mid_cb=None):
                b, qb = divmod(bi, NQB)
                q0 = b * T + qb * QB
                nkt = 4 * qb + 4
                psA0 = ppa.tile([65, QB], F32, name="psA0", tag="psA0")
                psA1 = ppa.tile([65, QB], F32, name="psA1", tag="psA1")
                for kl in range(nkt):
                    kt = b * NKT + kl
                    ksl = slice(kt * 128, (kt + 1) * 128)
                    v = kl - 4 * qb
                    qoff = max(v, 0) * 128
                    qsl = slice(q0 + qoff, q0 + QB)
                    psS = pps.tile([128, 2 * QB], F32, name="psS", tag="psS")
                    nc.tensor.matmul(psS[:, qoff:QB], kR[0:64, ksl],
                                     qR[0:64, qsl], start=True, stop=True,
                                     tile_position=(0, 0))
                    nc.tensor.matmul(psS[:, QB + qoff:2 * QB], kR[64:128, ksl],
                                     qR[64:128, qsl], start=True, stop=True,
                                     tile_position=(64, 0))
                    A = pbA.tile([128, 2 * QB], BF16, name="A", tag="A")
                    nc.scalar.activation(A[:, qoff:2 * QB], psS[:, qoff:2 * QB],
                                         ACT.Exp, bias=lnisx[:, kt:kt + 1],
                                         scale=ISQ)
                    if v >= 0:
                        nc.vector.tensor_tensor(A[:, qoff:qoff + 128],
                                                A[:, qoff:qoff + 128],
                                                umask[:], OP.mult)
                        nc.vector.tensor_tensor(A[:, QB + qoff:QB + qoff + 128],
                                                A[:, QB + qoff:QB + qoff + 128],
                                                umask[:], OP.mult)
                    st, sp = kl == 0, kl == nkt - 1
                    nc.tensor.matmul(psA0[:, qoff:QB],
                                     vaug[:, kt * VW:kt * VW + 65],
                                     A[:, qoff:QB], start=st, stop=sp)
                    nc.tensor.matmul(psA1[:, qoff:QB],
                                     vaug[:, kt * VW + 65:kt * VW + 130],
                                     A[:, QB + qoff:2 * QB], start=st, stop=sp)
                    if mid_cb is not None and kl == 7:
                        mid_cb()
                        mid_cb = None
                # evacuate numerators + denominator row
                nc.vector.tensor_copy(outU[0:65, q0:q0 + QB], psA0[:])
                nc.vector.tensor_copy(outU[0:65, NT + q0:NT + q0 + QB], psA1[:])
                # denominator reciprocal on 32 lanes
                rsq = pbq.tile([32, 32], F32, name="rsq", tag="rsq")
                nc.sync.dma_start(rsq[0:16, :], outU[64:65, q0:q0 + QB])
                nc.sync.dma_start(rsq[16:32, :], outU[64:65, NT + q0:NT + q0 + QB])
                rrec = pbq.tile([32, 32], F32, name="rrec", tag="rrec")
                nc.vector.reciprocal(rrec[:], rsq[:])
                nc.sync.dma_start(rinv[2 * bi:2 * bi + 1, :], rrec[0:16, :])
                nc.sync.dma_start(rinv[2 * bi + 1:2 * bi + 2, :], rrec[16:32, :])
                # renormalize in place
                for h in range(2):
                    r = 2 * bi + h
                    brs = ppb.tile([64, QB], F32, name="brs", tag="bb")
                    nc.tensor.matmul(brs[:], sel16[:, r * 64:(r + 1) * 64],
                                     r32(rinv[0:16, :]),
                                     start=True, stop=True)
                    colU = slice(h * NT + q0, h * NT + q0 + QB)
                    nc.vector.tensor_tensor(outU[0:64, colU], outU[0:64, colU],
                                            brs[:], OP.mult)
                # partial absmax over this core's 128 dims
                par0 = pb.tile([64, QB], F32, name="par0", tag="par0", bufs=1)
                nc.gpsimd.partition_all_reduce(par0[:], outU[0:64, q0:q0 + QB],
                                               channels=64,
                                               reduce_op=bass_isa.ReduceOp.absmax)
                par1 = pb.tile([64, QB], F32, name="par1", tag="par1", bufs=1)
                nc.gpsimd.partition_all_reduce(par1[:],
                                               outU[0:64, NT + q0:NT + q0 + QB],
                                               channels=64,
                                               reduce_op=bass_isa.ReduceOp.absmax)
                nc.vector.tensor_tensor(par0[0:1, :], par0[0:1, :],
                                        par1[0:1, :], OP.max)
                # ship renormalized dims + partial-max rows into the a2a
                # payload, split by token-owner quarter
                j = bi % NQB
                for half in range(2):
                    owner = 2 * j + half
                    tsl = slice(q0 + half * OT, q0 + (half + 1) * OT)
                    for h in range(2):
                        rr = PR * owner + 64 * h
                        nc.sync.dma_start(
                            a2a_in[b][rr:rr + 64, :],
                            outU[0:64, h * NT + q0 + half * OT:
                                 h * NT + q0 + (half + 1) * OT])
                    nc.sync.dma_start(a2a_in[b][PR * owner + 128:PR * owner + 129, :],
                                      par0[0:1, half * OT:(half + 1) * OT])

            def a2a_chunk(c):
                nc.gpsimd.collective_compute(
                    "AllToAll", OP.bypass, replica_groups=RG,
                    ins=[a2a_in[c][:].opt()], outs=[a2a_out[c][:].opt()])

            def owner_prep(c):
                """receive batch c: global scale, quantize."""
                gsb = []
                pm = pbq.tile([8, OT], F32, name="pm", tag="pm")
                for i in range(8):
                    g = pbq.tile([128, OT], F32, name=f"g{i}", tag=f"g{i}")
                    nc.sync.dma_start(g[:], a2a_out[c][i * PR:i * PR + 128, :])
                    gsb.append(g)
                    nc.sync.dma_start(pm[i:i + 1, :],
                                      a2a_out[c][i * PR + 128:(i + 1) * PR, :])
                pmr = pbq.tile([8, OT], F32, name="pmr", tag="pmr")
                nc.gpsimd.partition_all_reduce(pmr[:], pm[:], channels=8,
                                               reduce_op=bass_isa.ReduceOp.max)
                gm2 = pbq.tile([1, OT], F32, name="gm2", tag="gm2")
                nc.vector.tensor_scalar(gm2[:], pmr[0:1, :], 1e-5, None, OP.add)
                rec = pbq.tile([1, OT], F32, name="rec", tag="rec")
                nc.vector.reciprocal(rec[:], gm2[:])
                so_row = pbq.tile([1, OT], F32R, name="so_row", tag="so_row")
                nc.vector.tensor_scalar(so_row[:], rec[:], 127.0, None, OP.mult)
                iso_row = pbq.tile([1, OT], F32R, name="iso_row", tag="iso_row")
                nc.vector.tensor_scalar(iso_row[:], gm2[:], 1.0 / 127.0, None,
                                        OP.mult)
                sob = ppb.tile([128, OT], F32, name="sob", tag="bb")
                nc.tensor.matmul(sob[:], ones1[:], so_row[:],
                                 start=True, stop=True)
                sos = pb.tile([128, OT], F32, name="sos", tag="sos", bufs=1)
                nc.scalar.copy(sos[:], sob[:])
                isob = ppb.tile([128, OT], F32, name="isob", tag="bb")
                nc.tensor.matmul(isob[:], ones1[:], iso_row[:],
                                 start=True, stop=True)
                isos = pb.tile([128, OT], F32, name="isos", tag="isos", bufs=1)
                nc.scalar.copy(isos[:], isob[:])
                xq = []
                for i in range(8):
                    yq = pb.tile([128, OT], F32, name="yq", tag="yq")
                    nc.vector.tensor_tensor(yq[:], gsb[i][0:128, :], sos[:],
                                            OP.mult)
                    xqi = pbq.tile([128, OT], BF16, name=f"xq{i}", tag=f"xq{i}")
                    nc.vector.tensor_scalar(xqi[:], yq[:], RC, RC,
                                            OP.add, OP.subtract)
                    xq.append(xqi)
                return xq, isos

            def owner_wo(c, xq, isos):
                for oc in range(8):
                    pw = ppb.tile([128, OT], F32, name="pw", tag="bb")
                    for i in range(8):
                        nc.tensor.matmul(
                            pw[:],
                            w_sb[i][:, 3 * DPC + oc * 128:3 * DPC + (oc + 1) * 128],
                            xq[i][:], start=(i == 0), stop=(i == 7))
                    fin = pb.tile([128, OT], F32, name="fin", tag="fin")
                    nc.vector.tensor_tensor(fin[:], pw[:], isos[:], OP.mult)
                    nc.sync.dma_start(
                        out[oc * 128:(oc + 1) * 128, c * OT:(c + 1) * OT], fin[:])

            # Interleave: batch-1 projections fill the PE slack of batch-0's
            # exp-bound attention; the owner-side work for batch 0 runs in
            # the slack of blocks 5-7
            proj_pair(0)
            proj_pair(1)
            attention_block(0)
            attention_block(1)
            proj_pair(2)
            attention_block(2)
            proj_pair(3)
            attention_block(3)
            a2a_chunk(0)
            attention_block(4)
            attention_block(5)
            attention_block(6)
            st0 = owner_prep(0)
            attention_block(7, mid_cb=lambda: owner_wo(0, *st0))
            a2a_chunk(1)
            st1 = owner_prep(1)
            owner_wo(1, *st1)

    return nc


_CACHE = {}


def kernel(x, cos, sin, wq_w, wk_w, wv_w, wo_w):
    x = np.asarray(x, np.float32)
    cos = np.asarray(cos, np.float32)   # [T, 32]
    sin = np.asarray(sin, np.float32)
    xf = np.ascontiguousarray(x.reshape(NT, D))

    amax = np.abs(xf).max(-1) + 1e-5
    sx = (127.0 / amax).astype(np.float32)
    isx = (amax / 127.0).astype(np.float32)
    xq = np.clip(np.round(xf * sx[:, None]), -128.0, 127.0)
    xiT = np.ascontiguousarray(xq.T).astype(ml_dtypes.bfloat16)  # [D, NT]

    # RoPE tables (interleaved-pair convention) with isx folded in
    cm64 = np.repeat(cos.T, 2, axis=0)            # [64, T]
    sm64 = np.repeat(sin.T, 2, axis=0)
    cmap = np.tile(np.concatenate([cm64, cm64], axis=0), (1, B))
    smap = np.tile(np.concatenate([sm64, sm64], axis=0), (1, B))
    cmx = (cmap * isx[None, :]).astype(ml_dtypes.bfloat16)
    smx = (smap * isx[None, :]).astype(ml_dtypes.bfloat16)

    lnisx = np.ascontiguousarray(np.log(isx).reshape(NTT, 128).T).astype(np.float32)
    sxp = np.ascontiguousarray(sx.reshape(NTT, 128).T).astype(ml_dtypes.bfloat16)

    kk = np.arange(128)[:, None]
    jj = np.arange(128)[None, :]
    umask = (jj >= kk).astype(ml_dtypes.bfloat16)

    P = np.zeros((128, 128), np.float32)
    for j in range(64):
        P[2 * j, 2 * j + 1] = -1.0
        P[2 * j + 1, 2 * j] = 1.0
    pswapT = np.ascontiguousarray(P.T).astype(ml_dtypes.bfloat16)
    identb = np.eye(128, dtype=ml_dtypes.bfloat16)
    sel16 = np.zeros((16, 16 * 64), np.float32)
    for r in range(16):
        sel16[r, r * 64:(r + 1) * 64] = 1.0
    ones1 = np.ones((1, 128), np.float32)

    wq_e, wk_e, wv_e, wo_e = (_quant_w(np.asarray(w, np.float32))
                              for w in (wq_w, wk_w, wv_w, wo_w))
    woT = np.ascontiguousarray(wo_e.T)   # [D, D] full

    if "nc" not in _CACHE:
        nc0 = build_nc()
        nc0.finalize()
        _CACHE["nc"] = nc0
    nc = _CACHE["nc"]

    in_maps = []
    for c in range(N_CORES):
        hs = slice(c * DPC, (c + 1) * DPC)
        wall = np.concatenate(
            [np.ascontiguousarray(w[hs, :].T) for w in (wq_e, wk_e, wv_e)]
            + [woT], axis=1).astype(ml_dtypes.bfloat16)   # [D, 384+1024]
        in_maps.append({
            "xiT": xiT, "wall": wall, "cmx": cmx, "smx": smx,
            "lnisx": lnisx, "sxp": sxp, "umask": umask,
            "pswapT": pswapT, "identb": identb, "sel16": sel16, "ones1": ones1,
        })

    res = run_bass_kernel_spmd(nc, in_maps, core_ids=list(range(N_CORES)))
    outp = np.empty((NT, D), np.float32)
    for c in range(N_CORES):
        o = res.results[c]["out"]          # [D, 2*OT]
        outp[c * OT:(c + 1) * OT, :] = o[:, 0:OT].T
        outp[T + c * OT:T + (c + 1) * OT, :] = o[:, OT:2 * OT].T
    return outp.reshape(B, T, D)


# revision 47
# speedup vs baseline: 1.1462x; 1.1462x over previous
"""BitNet-style attention (B=2, T=2048, D=1024, 16 heads, RoPE, causal) on
8 TRN2 NeuronCores.

Head-parallel attention (2 heads/core) with a token-parallel wo:
  - host pre-quantizes x (int-valued bf16, transposed) and the ternary
    weights; per-token dequant scales (isx) are folded into the RoPE
    tables (q,k), the exp bias (A' = A*isx_k), and an augmented V column
    (1/isx_k = sx) that yields the softmax denominator for free.
  - scores: 2 heads row-packed on the PE (K=64 each); causal blocks get
    a restricted moving dim; the 128-wide diagonal is masked post-exp.
  - batch-1 projections are interleaved into batch-0's (exp-bound)
    attention stream through a shared PSUM pool set.
  - one AllToAll per batch reshards the renormalized f32 attention out
    by token owner (128 dims + a partial-absmax row per core); owners
    compute the int8 scale, quantize, and their 512-token slice of wo
    locally -- no AllReduce / AllGather on the critical path.
"""

import math
from contextlib import ExitStack

import ml_dtypes
import numpy as np

import concourse.bass as bass
import concourse.bacc as bacc_mod
import concourse.bass_isa as bass_isa
import concourse.mybir as mybir
import concourse.tile as tile
from concourse.bass_utils import run_bass_kernel_spmd

F32 = mybir.dt.float32
F32R = mybir.dt.float32r
BF16 = mybir.dt.bfloat16
OP = mybir.AluOpType
ACT = mybir.ActivationFunctionType

B, T, D = 2, 2048, 1024
NT = B * T              # 4096 tokens
NH, HD = 16, 64
N_CORES = 8
HPC = NH // N_CORES     # heads/core = 2
DPC = HPC * HD          # dims/core = 128
RC = 12582912.0         # 1.5*2^23 round-to-nearest-even constant

TB = 512                # token block (matmul N)
NTB = NT // TB          # 8
NTT = NT // 128         # 32 token tiles
QB = 512                # q block
NQB = T // QB           # 4 per batch
NKT = T // 128          # 16 k tiles per batch
VW = 130                # vaug group width: [v_h0(64) | sx | v_h1(64) | sx]
OT = T // N_CORES       # tokens owned per core per batch = 256
PR = 129                # a2a part rows: 128 dims + 1 partial-max row
ISQ = 1.0 / math.sqrt(HD)


def _quant_w(w):
    O, I = w.shape
    wg = w.reshape(O, I // 128, 128)
    ws = np.abs(wg).mean(-1, keepdims=True) + 1e-5
    wq = np.clip(np.round(wg / ws), -1.0, 1.0) * ws
    return wq.reshape(O, I).astype(np.float32)


def build_nc():
    nc = bacc_mod.Bacc(num_devices=N_CORES)
    io = {}

    def inp(name, shape, dt=F32):
        io[name] = nc.dram_tensor(name, shape, dt, kind="ExternalInput")

    inp("xiT", [D, NT], BF16)        # quantized x, transposed (int-valued)
    inp("wall", [D, 3 * DPC + D], BF16)  # [wq|wk|wv slices | full woT]
    inp("cmx", [128, NT], BF16)      # cos table * isx
    inp("smx", [128, NT], BF16)      # sin table * isx
    inp("lnisx", [128, NTT], F32)    # ln(isx) laid out [token%128, tile]
    inp("sxp", [128, NTT], BF16)     # sx laid out [token%128, tile]
    inp("umask", [128, 128], BF16)   # tri mask (1 if q>=k)
    inp("pswapT", [128, 128], BF16)  # RoPE pair swap
    inp("identb", [128, 128], BF16)
    inp("sel16", [16, 16 * 64], F32R)   # one-hot row selectors (renorm bcast)
    inp("ones1", [1, 128], F32R)
    out = nc.dram_tensor("out", [D, 2 * OT], F32, kind="ExternalOutput")

    r32 = lambda ap: ap.bitcast(F32R)
    RG = [list(range(N_CORES))]

    with nc.allow_low_precision(reason="bf16 matmul pipeline on int-exact activations"), \
         tile.TileContext(nc) as tc, ExitStack() as top:
        cpool = top.enter_context(tc.tile_pool(name="const", bufs=1))
        dpool = top.enter_context(tc.tile_pool(name="dram", bufs=1, space="DRAM"))

        # ---------------- persistent tiles ----------------
        w_sb = [cpool.tile([128, 3 * DPC + D], BF16, name=f"w{i}", tag=f"w{i}")
                for i in range(8)]
        xi_sb = [cpool.tile([128, NT], BF16, name=f"xi{i}", tag=f"xi{i}")
                 for i in range(8)]
        lnisx = cpool.tile([128, NTT], F32, name="lnisx", tag="lnisx")
        sxp = cpool.tile([128, NTT], BF16, name="sxp", tag="sxp")
        umask = cpool.tile([128, 128], BF16, name="umask", tag="umask")
        pswapT = cpool.tile([128, 128], BF16, name="pswapT", tag="pswapT")
        identb = cpool.tile([128, 128], BF16, name="identb", tag="identb")
        sel16 = cpool.tile([16, 16 * 64], F32R, name="sel16", tag="sel16")
        ones1 = cpool.tile([1, 128], F32R, name="ones1", tag="ones1")

        qR = cpool.tile([128, NT], BF16, name="qR", tag="qR")
        kR = cpool.tile([128, NT], BF16, name="kR", tag="kR")
        vaug = cpool.tile([128, NTT * VW], BF16, name="vaug", tag="vaug")
        # unnormalized attention out + denominator row (row 64);
        # head0 cols [0:NT), head1 cols [NT:2NT)
        outU = cpool.tile([65, 2 * NT], F32, name="outU", tag="outU")
        rinv = cpool.tile([16, QB], F32, name="rinv", tag="rinv")

        # DRAM scratch: AllToAll payload = 8 parts x (128 dim rows + 1
        # partial-max row) x 256 owned tokens, f32
        a2a_in = [dpool.tile([8 * PR, OT], F32, name=f"a2a_in{c}",
                             tag=f"a2a_in{c}") for c in range(B)]
        a2a_out = [dpool.tile([8 * PR, OT], F32, name=f"a2a_out{c}",
                              tag=f"a2a_out{c}") for c in range(B)]

        # input DMAs, all on the sync ring, in consumption order
        nc.sync.dma_start(identb[:], io["identb"][:])
        for nm, t in (("lnisx", lnisx), ("sxp", sxp), ("umask", umask),
                      ("pswapT", pswapT), ("sel16", sel16), ("ones1", ones1)):
            nc.sync.dma_start(t[:], io[nm][:])
        for i in range(8):
            nc.sync.dma_start(w_sb[i][:, 0:3 * DPC],
                              io["wall"][i * 128:(i + 1) * 128, 0:3 * DPC])
        # staged rows are contracted against one-hot selectors before every
        # row is written; zero-init so 0*garbage can't produce NaN
        nc.vector.memset(rinv[:], 0.0)
        # x in 256 KB slices, token-pair-major
        for tp in range(4):
            sl = slice(tp * 1024, (tp + 1) * 1024)
            for i in range(8):
                nc.sync.dma_start(xi_sb[i][:, sl],
                                  io["xiT"][i * 128:(i + 1) * 128, sl])
        # wo weights are needed last
        for i in range(8):
            nc.sync.dma_start(w_sb[i][:, 3 * DPC:],
                              io["wall"][i * 128:(i + 1) * 128, 3 * DPC:])

        # PE warm-up: ~13 us of dependency-free matmuls so the HAM clock
        # gate opens while the input DMAs stream in
        with tc.tile_pool(name="pwm", bufs=1, space="PSUM") as pwm:
            warm = pwm.tile([128, 128], F32, name="warm", tag="warm")
            for _ in range(120):
                nc.tensor.matmul(warm[:], identb[:], identb[:],
                                 start=True, stop=True)

        # prefill vaug sx columns (cols 64 and 129 of each 130-wide group)
        for kt in range(NTT):
            nc.vector.tensor_copy(vaug[:, kt * VW + 64:kt * VW + 65],
                                  sxp[:, kt:kt + 1])
            nc.vector.tensor_copy(vaug[:, kt * VW + 129:kt * VW + 130],
                                  sxp[:, kt:kt + 1])

        # One shared PSUM pool set for both phases so their emission can be
        # interleaved: psS 2x[128,1024] (4 banks) + psA0/psA1 (2) + bb
        # 2x[128,512] (2) = 8 banks
        with tc.tile_pool(name="pps", bufs=2, space="PSUM") as pps, \
             tc.tile_pool(name="ppa", bufs=1, space="PSUM") as ppa, \
             tc.tile_pool(name="ppb", bufs=2, space="PSUM") as ppb, \
             tc.tile_pool(name="pa", bufs=2) as pa, \
             tc.tile_pool(name="pb", bufs=2) as pb, \
             tc.tile_pool(name="pbA", bufs=2) as pbA, \
             tc.tile_pool(name="pbq", bufs=1) as pbq:

            def proj_pair(tp):
                """qkv + RoPE + V transpose for token blocks 2tp, 2tp+1."""
                tbs = (2 * tp, 2 * tp + 1)
                sls = [slice(tb * TB, (tb + 1) * TB) for tb in tbs]
                cmb = pa.tile([128, 2 * TB], BF16, name="cmb", tag="cmb")
                nc.sync.dma_start(cmb[:], io["cmx"][:, tp * 1024:(tp + 1) * 1024])
                smb = pa.tile([128, 2 * TB], BF16, name="smb", tag="smb")
                nc.sync.dma_start(smb[:], io["smx"][:, tp * 1024:(tp + 1) * 1024])
                blk = {}
                for pi, pname in enumerate(("q", "k", "v")):
                    pp2 = pps.tile([128, 2 * TB], F32, name="pp2", tag="psS")
                    for i in range(8):
                        for u in range(2):
                            nc.tensor.matmul(pp2[:, u * TB:(u + 1) * TB],
                                             w_sb[i][:, pi * 128:(pi + 1) * 128],
                                             xi_sb[i][:, sls[u]],
                                             start=(i == 0), stop=(i == 7))
                    for u in range(2):
                        t = pa.tile([128, TB], BF16, name=f"t_{pname}{u}",
                                    tag=f"t_{pname}{u}")
                        if pname == "v":
                            nc.vector.tensor_copy(t[:], pp2[:, u * TB:(u + 1) * TB])
                        else:
                            nc.scalar.copy(t[:], pp2[:, u * TB:(u + 1) * TB])
                        blk[(pname, u)] = t
                for u in range(2):
                    sl = sls[u]
                    csl = slice(u * TB, (u + 1) * TB)
                    for pname, dstR in (("q", qR), ("k", kR)):
                        src = blk[(pname, u)]
                        swp = ppb.tile([128, TB], F32, name="swp", tag="bb")
                        nc.tensor.matmul(swp[:], pswapT[:], src[:],
                                         start=True, stop=True)
                        t1 = pa.tile([128, TB], BF16, name="t1", tag="t1")
                        nc.vector.tensor_tensor(t1[:], src[:], cmb[:, csl], OP.mult)
                        t2 = pa.tile([128, TB], BF16, name="t2", tag="t2")
                        nc.vector.tensor_tensor(t2[:], swp[:], smb[:, csl], OP.mult)
                        nc.vector.tensor_tensor(dstR[:, sl], t1[:], t2[:], OP.add)
                    for j in range(TB // 128):
                        kt = tbs[u] * 4 + j
                        vtp = ppb.tile([128, 128], BF16, name="vtp", tag="bb")
                        nc.tensor.transpose(vtp[:],
                                            blk[("v", u)][:, j * 128:(j + 1) * 128],
                                            identb[:])
                        nc.vector.tensor_copy(vaug[:, kt * VW:kt * VW + 64],
                                              vtp[:, 0:64])
                        nc.vector.tensor_copy(vaug[:, kt * VW + 65:kt * VW + 129],
                                              vtp[:, 64:128])

            def attention_block(bi):
                b, qb = divmod(bi, NQB)
                q0 = b * T + qb * QB
                nkt = 4 * qb + 4
                psA0 = ppa.tile([65, QB], F32, name="psA0", tag="psA0")
                psA1 = ppa.tile([65, QB], F32, name="psA1", tag="psA1")
                for kl in range(nkt):
                    kt = b * NKT + kl
                    ksl = slice(kt * 128, (kt + 1) * 128)
                    v = kl - 4 * qb
                    qoff = max(v, 0) * 128
                    qsl = slice(q0 + qoff, q0 + QB)
                    psS = pps.tile([128, 2 * QB], F32, name="psS", tag="psS")
                    nc.tensor.matmul(psS[:, qoff:QB], kR[0:64, ksl],
                                     qR[0:64, qsl], start=True, stop=True,
                                     tile_position=(0, 0))
                    nc.tensor.matmul(psS[:, QB + qoff:2 * QB], kR[64:128, ksl],
                                     qR[64:128, qsl], start=True, stop=True,
                                     tile_position=(64, 0))
                    A = pbA.tile([128, 2 * QB], BF16, name="A", tag="A")
                    nc.scalar.activation(A[:, qoff:2 * QB], psS[:, qoff:2 * QB],
                                         ACT.Exp, bias=lnisx[:, kt:kt + 1],
                                         scale=ISQ)
                    if v >= 0:
                        nc.vector.tensor_tensor(A[:, qoff:qoff + 128],
                                                A[:, qoff:qoff + 128],
                                                umask[:], OP.mult)
                        nc.vector.tensor_tensor(A[:, QB + qoff:QB + qoff + 128],
                                                A[:, QB + qoff:QB + qoff + 128],
                                                umask[:], OP.mult)
                    st, sp = kl == 0, kl == nkt - 1
                    nc.tensor.matmul(psA0[:, qoff:QB],
                                     vaug[:, kt * VW:kt * VW + 65],
                                     A[:, qoff:QB], start=st, stop=sp)
                    nc.tensor.matmul(psA1[:, qoff:QB],
                                     vaug[:, kt * VW + 65:kt * VW + 130],
                                     A[:, QB + qoff:2 * QB], start=st, stop=sp)
                # evacuate numerators + denominator row
                nc.vector.tensor_copy(outU[0:65, q0:q0 + QB], psA0[:])
                nc.vector.tensor_copy(outU[0:65, NT + q0:NT + q0 + QB], psA1[:])
                # denominator reciprocal on 32 lanes
                rsq = pbq.tile([32, 32], F32, name="rsq", tag="rsq")
                nc.sync.dma_start(rsq[0:16, :], outU[64:65, q0:q0 + QB])
                nc.sync.dma_start(rsq[16:32, :], outU[64:65, NT + q0:NT + q0 + QB])
                rrec = pbq.tile([32, 32], F32, name="rrec", tag="rrec")
                nc.vector.reciprocal(rrec[:], rsq[:])
                nc.sync.dma_start(rinv[2 * bi:2 * bi + 1, :], rrec[0:16, :])
                nc.sync.dma_start(rinv[2 * bi + 1:2 * bi + 2, :], rrec[16:32, :])
                # renormalize in place
                for h in range(2):
                    r = 2 * bi + h
                    brs = ppb.tile([64, QB], F32, name="brs", tag="bb")
                    nc.tensor.matmul(brs[:], sel16[:, r * 64:(r + 1) * 64],
                                     r32(rinv[0:16, :]),
                                     start=True, stop=True)
                    colU = slice(h * NT + q0, h * NT + q0 + QB)
                    nc.vector.tensor_tensor(outU[0:64, colU], outU[0:64, colU],
                                            brs[:], OP.mult)
                # partial absmax over this core's 128 dims
                par0 = pb.tile([64, QB], F32, name="par0", tag="par0", bufs=1)
                nc.gpsimd.partition_all_reduce(par0[:], outU[0:64, q0:q0 + QB],
                                               channels=64,
                                               reduce_op=bass_isa.ReduceOp.absmax)
                par1 = pb.tile([64, QB], F32, name="par1", tag="par1", bufs=1)
                nc.gpsimd.partition_all_reduce(par1[:],
                                               outU[0:64, NT + q0:NT + q0 + QB],
                                               channels=64,
                                               reduce_op=bass_isa.ReduceOp.absmax)
                nc.vector.tensor_tensor(par0[0:1, :], par0[0:1, :],
                                        par1[0:1, :], OP.max)
                # ship renormalized dims + partial-max rows into the a2a
                # payload, split by token-owner quarter
                j = bi % NQB
                for half in range(2):
                    owner = 2 * j + half
                    tsl = slice(q0 + half * OT, q0 + (half + 1) * OT)
                    for h in range(2):
                        rr = PR * owner + 64 * h
                        nc.sync.dma_start(
                            a2a_in[b][rr:rr + 64, :],
                            outU[0:64, h * NT + q0 + half * OT:
                                 h * NT + q0 + (half + 1) * OT])
                    nc.sync.dma_start(a2a_in[b][PR * owner + 128:PR * owner + 129, :],
                                      par0[0:1, half * OT:(half + 1) * OT])

            def a2a_chunk(c):
                nc.gpsimd.collective_compute(
                    "AllToAll", OP.bypass, replica_groups=RG,
                    ins=[a2a_in[c][:].opt()], outs=[a2a_out[c][:].opt()])

            def owner_chunk(c):
                """receive batch c: global scale, quantize, wo, output."""
                gsb = []
                pm = pbq.tile([8, OT], F32, name="pm", tag="pm")
                for i in range(8):
                    g = pbq.tile([128, OT], F32, name=f"g{i}", tag=f"g{i}")
                    nc.sync.dma_start(g[:], a2a_out[c][i * PR:i * PR + 128, :])
                    gsb.append(g)
                    nc.sync.dma_start(pm[i:i + 1, :],
                                      a2a_out[c][i * PR + 128:(i + 1) * PR, :])
                pmr = pbq.tile([8, OT], F32, name="pmr", tag="pmr")
                nc.gpsimd.partition_all_reduce(pmr[:], pm[:], channels=8,
                                               reduce_op=bass_isa.ReduceOp.max)
                gm2 = pbq.tile([1, OT], F32, name="gm2", tag="gm2")
                nc.vector.tensor_scalar(gm2[:], pmr[0:1, :], 1e-5, None, OP.add)
                rec = pbq.tile([1, OT], F32, name="rec", tag="rec")
                nc.vector.reciprocal(rec[:], gm2[:])
                so_row = pbq.tile([1, OT], F32R, name="so_row", tag="so_row")
                nc.vector.tensor_scalar(so_row[:], rec[:], 127.0, None, OP.mult)
                iso_row = pbq.tile([1, OT], F32R, name="iso_row", tag="iso_row")
                nc.vector.tensor_scalar(iso_row[:], gm2[:], 1.0 / 127.0, None,
                                        OP.mult)
                sob = ppb.tile([128, OT], F32, name="sob", tag="bb")
                nc.tensor.matmul(sob[:], ones1[:], so_row[:],
                                 start=True, stop=True)
                sos = pb.tile([128, OT], F32, name="sos", tag="sos", bufs=1)
                nc.scalar.copy(sos[:], sob[:])
                isob = ppb.tile([128, OT], F32, name="isob", tag="bb")
                nc.tensor.matmul(isob[:], ones1[:], iso_row[:],
                                 start=True, stop=True)
                isos = pb.tile([128, OT], F32, name="isos", tag="isos", bufs=1)
                nc.scalar.copy(isos[:], isob[:])
                xq = []
                for i in range(8):
                    yq = pb.tile([128, OT], F32, name="yq", tag="yq")
                    nc.vector.tensor_tensor(yq[:], gsb[i][0:128, :], sos[:],
                                            OP.mult)
                    xqi = pbq.tile([128, OT], BF16, name=f"xq{i}", tag=f"xq{i}")
                    nc.vector.tensor_scalar(xqi[:], yq[:], RC, RC,
                                            OP.add, OP.subtract)
                    xq.append(xqi)
                for oc in range(8):
                    pw = ppb.tile([128, OT], F32, name="pw", tag="bb")
                    for i in range(8):
                        nc.tensor.matmul(
                            pw[:],
                            w_sb[i][:, 3 * DPC + oc * 128:3 * DPC + (oc + 1) * 128],
                            xq[i][:], start=(i == 0), stop=(i == 7))
                    fin = pb.tile([128, OT], F32, name="fin", tag="fin")
                    nc.vector.tensor_tensor(fin[:], pw[:], isos[:], OP.mult)
                    nc.sync.dma_start(
                        out[oc * 128:(oc + 1) * 128, c * OT:(c + 1) * OT], fin[:])

            # Interleave: batch-1 projections fill the PE slack of batch-0's
            # exp-bound attention; the owner-side work for batch 0 runs in
            # the slack of blocks 5-7
            proj_pair(0)
            proj_pair(1)
            attention_block(0)
            attention_block(1)
            proj_pair(2)
            attention_block(2)
            proj_pair(3)
            attention_block(3)
            a2a_chunk(0)
            attention_block(4)
            attention_block(5)
            attention_block(6)
            owner_chunk(0)
            attention_block(7)
            a2a_chunk(1)
            owner_chunk(1)

    return nc


_CACHE = {}


def kernel(x, cos, sin, wq_w, wk_w, wv_w, wo_w):
    x = np.asarray(x, np.float32)
    cos = np.asarray(cos, np.float32)   # [T, 32]
    sin = np.asarray(sin, np.float32)
    xf = np.ascontiguousarray(x.reshape(NT, D))

    amax = np.abs(xf).max(-1) + 1e-5
    sx = (127.0 / amax).astype(np.float32)
    isx = (amax / 127.0).astype(np.float32)
    xq = np.clip(np.round(xf * sx[:, None]), -128.0, 127.0)
    xiT = np.ascontiguousarray(xq.T).astype(ml_dtypes.bfloat16)  # [D, NT]

    # RoPE tables (interleaved-pair convention) with isx folded in
    cm64 = np.repeat(cos.T, 2, axis=0)            # [64, T]
    sm64 = np.repeat(sin.T, 2, axis=0)
    cmap = np.tile(np.concatenate([cm64, cm64], axis=0), (1, B))
    smap = np.tile(np.concatenate([sm64, sm64], axis=0), (1, B))
    cmx = (cmap * isx[None, :]).astype(ml_dtypes.bfloat16)
    smx = (smap * isx[None, :]).astype(ml_dtypes.bfloat16)

    lnisx = np.ascontiguousarray(np.log(isx).reshape(NTT, 128).T).astype(np.float32)
    sxp = np.ascontiguousarray(sx.reshape(NTT, 128).T).astype(ml_dtypes.bfloat16)

    kk = np.arange(128)[:, None]
    jj = np.arange(128)[None, :]
    umask = (jj >= kk).astype(ml_dtypes.bfloat16)

    P = np.zeros((128, 128), np.float32)
    for j in range(64):
        P[2 * j, 2 * j + 1] = -1.0
        P[2 * j + 1, 2 * j] = 1.0
    pswapT = np.ascontiguousarray(P.T).astype(ml_dtypes.bfloat16)
    identb = np.eye(128, dtype=ml_dtypes.bfloat16)
    sel16 = np.zeros((16, 16 * 64), np.float32)
    for r in range(16):
        sel16[r, r * 64:(r + 1) * 64] = 1.0
    ones1 = np.ones((1, 128), np.float32)

    wq_e, wk_e, wv_e, wo_e = (_quant_w(np.asarray(w, np.float32))
                              for w in (wq_w, wk_w, wv_w, wo_w))
    woT = np.ascontiguousarray(wo_e.T)   # [D, D] full

    if "nc" not in _CACHE:
        nc0 = build_nc()
        nc0.finalize()
        _CACHE["nc"] = nc0
    nc = _CACHE["nc"]

    in_maps = []
    for c in range(N_CORES):
        hs = slice(c * DPC, (c + 1) * DPC)
        wall = np.concatenate(
            [np.ascontiguousarray(w[hs, :].T) for w in (wq_e, wk_e, wv_e)]
            + [woT], axis=1).astype(ml_dtypes.bfloat16)   # [D, 384+1024]
        in_maps.append({
            "xiT": xiT, "wall": wall, "cmx": cmx, "smx": smx,
            "lnisx": lnisx, "sxp": sxp, "umask": umask,
            "pswapT": pswapT, "identb": identb, "sel16": sel16, "ones1": ones1,
        })

    res = run_bass_kernel_spmd(nc, in_maps, core_ids=list(range(N_CORES)))
    outp = np.empty((NT, D), np.float32)
    for c in range(N_CORES):
        o = res.results[c]["out"]          # [D, 2*OT]
        outp[c * OT:(c + 1) * OT, :] = o[:, 0:OT].T
        outp[T + c * OT:T + (c + 1) * OT, :] = o[:, OT:2 * OT].T
    return outp.reshape(B, T, D)


# revision 48
# speedup vs baseline: 1.1562x; 1.0087x over previous
"""BitNet-style attention (B=2, T=2048, D=1024, 16 heads, RoPE, causal) on
8 TRN2 NeuronCores.

Head-parallel attention (2 heads/core) with a token-parallel wo:
  - host pre-quantizes x (int-valued bf16, transposed) and the ternary
    weights; per-token dequant scales (isx) are folded into the RoPE
    tables (q,k), the exp bias (A' = A*isx_k), and an augmented V column
    (1/isx_k = sx) that yields the softmax denominator for free.
  - scores: 2 heads row-packed on the PE (K=64 each); causal blocks get
    a restricted moving dim; the 128-wide diagonal is masked post-exp.
  - batch-1 projections are interleaved into batch-0's (exp-bound)
    attention stream through a shared PSUM pool set.
  - one AllToAll per batch reshards the renormalized f32 attention out
    by token owner (128 dims + a partial-absmax row per core); owners
    compute the int8 scale, quantize, and their 512-token slice of wo
    locally -- no AllReduce / AllGather on the critical path.
"""

import math
from contextlib import ExitStack

import ml_dtypes
import numpy as np

import concourse.bass as bass
import concourse.bacc as bacc_mod
import concourse.bass_isa as bass_isa
import concourse.mybir as mybir
import concourse.tile as tile
from concourse.bass_utils import run_bass_kernel_spmd

F32 = mybir.dt.float32
F32R = mybir.dt.float32r
BF16 = mybir.dt.bfloat16
OP = mybir.AluOpType
ACT = mybir.ActivationFunctionType

B, T, D = 2, 2048, 1024
NT = B * T              # 4096 tokens
NH, HD = 16, 64
N_CORES = 8
HPC = NH // N_CORES     # heads/core = 2
DPC = HPC * HD          # dims/core = 128
RC = 12582912.0         # 1.5*2^23 round-to-nearest-even constant

TB = 512                # token block (matmul N)
NTB = NT // TB          # 8
NTT = NT // 128         # 32 token tiles
QB = 512                # q block
NQB = T // QB           # 4 per batch
NKT = T // 128          # 16 k tiles per batch
VW = 130                # vaug group width: [v_h0(64) | sx | v_h1(64) | sx]
OT = T // N_CORES       # tokens owned per core per batch = 256
PR = 129                # a2a part rows: 128 dims + 1 partial-max row
ISQ = 1.0 / math.sqrt(HD)


def _quant_w(w):
    O, I = w.shape
    wg = w.reshape(O, I // 128, 128)
    ws = np.abs(wg).mean(-1, keepdims=True) + 1e-5
    wq = np.clip(np.round(wg / ws), -1.0, 1.0) * ws
    return wq.reshape(O, I).astype(np.float32)


def build_nc():
    nc = bacc_mod.Bacc(num_devices=N_CORES)
    io = {}

    def inp(name, shape, dt=F32):
        io[name] = nc.dram_tensor(name, shape, dt, kind="ExternalInput")

    inp("xiT", [D, NT], BF16)        # quantized x, transposed (int-valued)
    inp("wall", [D, 3 * DPC + D], BF16)  # [wq|wk|wv slices | full woT]
    inp("cmx", [128, NT], BF16)      # cos table * isx
    inp("smx", [128, NT], BF16)      # sin table * isx
    inp("lnisx", [128, NTT], F32)    # ln(isx) laid out [token%128, tile]
    inp("sxp", [128, NTT], BF16)     # sx laid out [token%128, tile]
    inp("umask", [128, 128], BF16)   # tri mask (1 if q>=k)
    inp("pswapT", [128, 128], BF16)  # RoPE pair swap
    inp("identb", [128, 128], BF16)
    inp("sel16", [16, 16 * 64], F32R)   # one-hot row selectors (renorm bcast)
    inp("ones1", [1, 128], F32R)
    out = nc.dram_tensor("out", [D, 2 * OT], F32, kind="ExternalOutput")

    r32 = lambda ap: ap.bitcast(F32R)
    RG = [list(range(N_CORES))]

    with nc.allow_low_precision(reason="bf16 matmul pipeline on int-exact activations"), \
         tile.TileContext(nc) as tc, ExitStack() as top:
        cpool = top.enter_context(tc.tile_pool(name="const", bufs=1))
        dpool = top.enter_context(tc.tile_pool(name="dram", bufs=1, space="DRAM"))

        # ---------------- persistent tiles ----------------
        w_sb = [cpool.tile([128, 3 * DPC + D], BF16, name=f"w{i}", tag=f"w{i}")
                for i in range(8)]
        xi_sb = [cpool.tile([128, NT], BF16, name=f"xi{i}", tag=f"xi{i}")
                 for i in range(8)]
        lnisx = cpool.tile([128, NTT], F32, name="lnisx", tag="lnisx")
        sxp = cpool.tile([128, NTT], BF16, name="sxp", tag="sxp")
        umask = cpool.tile([128, 128], BF16, name="umask", tag="umask")
        pswapT = cpool.tile([128, 128], BF16, name="pswapT", tag="pswapT")
        identb = cpool.tile([128, 128], BF16, name="identb", tag="identb")
        sel16 = cpool.tile([16, 16 * 64], F32R, name="sel16", tag="sel16")
        ones1 = cpool.tile([1, 128], F32R, name="ones1", tag="ones1")

        qR = cpool.tile([128, NT], BF16, name="qR", tag="qR")
        kR = cpool.tile([128, NT], BF16, name="kR", tag="kR")
        vaug = cpool.tile([128, NTT * VW], BF16, name="vaug", tag="vaug")
        # unnormalized attention out + denominator row (row 64);
        # head0 cols [0:NT), head1 cols [NT:2NT)
        outU = cpool.tile([65, 2 * NT], F32, name="outU", tag="outU")
        rinv = cpool.tile([16, QB], F32, name="rinv", tag="rinv")

        # DRAM scratch: AllToAll payload = 8 parts x (128 dim rows + 1
        # partial-max row) x 256 owned tokens, f32
        a2a_in = [dpool.tile([8 * PR, OT], F32, name=f"a2a_in{c}",
                             tag=f"a2a_in{c}") for c in range(B)]
        a2a_out = [dpool.tile([8 * PR, OT], F32, name=f"a2a_out{c}",
                              tag=f"a2a_out{c}") for c in range(B)]

        # input DMAs, all on the sync ring, in consumption order
        nc.sync.dma_start(identb[:], io["identb"][:])
        for nm, t in (("lnisx", lnisx), ("sxp", sxp), ("umask", umask),
                      ("pswapT", pswapT), ("sel16", sel16), ("ones1", ones1)):
            nc.sync.dma_start(t[:], io[nm][:])
        for i in range(8):
            nc.sync.dma_start(w_sb[i][:, 0:3 * DPC],
                              io["wall"][i * 128:(i + 1) * 128, 0:3 * DPC])
        # staged rows are contracted against one-hot selectors before every
        # row is written; zero-init so 0*garbage can't produce NaN
        nc.vector.memset(rinv[:], 0.0)
        # x in 256 KB slices, token-pair-major
        for tp in range(4):
            sl = slice(tp * 1024, (tp + 1) * 1024)
            for i in range(8):
                nc.sync.dma_start(xi_sb[i][:, sl],
                                  io["xiT"][i * 128:(i + 1) * 128, sl])
        # PE warm-up: ~13 us of dependency-free matmuls so the HAM clock
        # gate opens while the input DMAs stream in
        with tc.tile_pool(name="pwm", bufs=1, space="PSUM") as pwm:
            warm = pwm.tile([128, 128], F32, name="warm", tag="warm")
            for _ in range(120):
                nc.tensor.matmul(warm[:], identb[:], identb[:],
                                 start=True, stop=True)

        # prefill vaug sx columns (cols 64 and 129 of each 130-wide group)
        for kt in range(NTT):
            nc.vector.tensor_copy(vaug[:, kt * VW + 64:kt * VW + 65],
                                  sxp[:, kt:kt + 1])
            nc.vector.tensor_copy(vaug[:, kt * VW + 129:kt * VW + 130],
                                  sxp[:, kt:kt + 1])

        # One shared PSUM pool set for both phases so their emission can be
        # interleaved: psS 2x[128,1024] (4 banks) + psA0/psA1 (2) + bb
        # 2x[128,512] (2) = 8 banks
        with tc.tile_pool(name="pps", bufs=2, space="PSUM") as pps, \
             tc.tile_pool(name="ppa", bufs=1, space="PSUM") as ppa, \
             tc.tile_pool(name="ppb", bufs=2, space="PSUM") as ppb, \
             tc.tile_pool(name="pa", bufs=2) as pa, \
             tc.tile_pool(name="pb", bufs=2) as pb, \
             tc.tile_pool(name="pbA", bufs=3) as pbA, \
             tc.tile_pool(name="pbq", bufs=1) as pbq:

            def proj_pair(tp):
                """qkv + RoPE + V transpose for token blocks 2tp, 2tp+1."""
                tbs = (2 * tp, 2 * tp + 1)
                sls = [slice(tb * TB, (tb + 1) * TB) for tb in tbs]
                cmb = pa.tile([128, 2 * TB], BF16, name="cmb", tag="cmb")
                nc.sync.dma_start(cmb[:], io["cmx"][:, tp * 1024:(tp + 1) * 1024])
                smb = pa.tile([128, 2 * TB], BF16, name="smb", tag="smb")
                nc.sync.dma_start(smb[:], io["smx"][:, tp * 1024:(tp + 1) * 1024])
                blk = {}
                for pi, pname in enumerate(("q", "k", "v")):
                    pp2 = pps.tile([128, 2 * TB], F32, name="pp2", tag="psS")
                    for i in range(8):
                        for u in range(2):
                            nc.tensor.matmul(pp2[:, u * TB:(u + 1) * TB],
                                             w_sb[i][:, pi * 128:(pi + 1) * 128],
                                             xi_sb[i][:, sls[u]],
                                             start=(i == 0), stop=(i == 7))
                    for u in range(2):
                        t = pa.tile([128, TB], BF16, name=f"t_{pname}{u}",
                                    tag=f"t_{pname}{u}")
                        if pname == "v":
                            nc.vector.tensor_copy(t[:], pp2[:, u * TB:(u + 1) * TB])
                        else:
                            nc.scalar.copy(t[:], pp2[:, u * TB:(u + 1) * TB])
                        blk[(pname, u)] = t
                for u in range(2):
                    sl = sls[u]
                    csl = slice(u * TB, (u + 1) * TB)
                    for pname, dstR in (("q", qR), ("k", kR)):
                        src = blk[(pname, u)]
                        swp = ppb.tile([128, TB], F32, name="swp", tag="bb")
                        nc.tensor.matmul(swp[:], pswapT[:], src[:],
                                         start=True, stop=True)
                        t1 = pa.tile([128, TB], BF16, name="t1", tag="t1")
                        nc.vector.tensor_tensor(t1[:], src[:], cmb[:, csl], OP.mult)
                        t2 = pa.tile([128, TB], BF16, name="t2", tag="t2")
                        nc.vector.tensor_tensor(t2[:], swp[:], smb[:, csl], OP.mult)
                        nc.vector.tensor_tensor(dstR[:, sl], t1[:], t2[:], OP.add)
                    for j in range(TB // 128):
                        kt = tbs[u] * 4 + j
                        vtp = ppb.tile([128, 128], BF16, name="vtp", tag="bb")
                        nc.tensor.transpose(vtp[:],
                                            blk[("v", u)][:, j * 128:(j + 1) * 128],
                                            identb[:])
                        nc.vector.tensor_copy(vaug[:, kt * VW:kt * VW + 64],
                                              vtp[:, 0:64])
                        nc.vector.tensor_copy(vaug[:, kt * VW + 65:kt * VW + 129],
                                              vtp[:, 64:128])

            def attention_block(bi):
                b, qb = divmod(bi, NQB)
                q0 = b * T + qb * QB
                nkt = 4 * qb + 4
                psA0 = ppa.tile([65, QB], F32, name="psA0", tag="psA0")
                psA1 = ppa.tile([65, QB], F32, name="psA1", tag="psA1")
                for kl in range(nkt):
                    kt = b * NKT + kl
                    ksl = slice(kt * 128, (kt + 1) * 128)
                    v = kl - 4 * qb
                    qoff = max(v, 0) * 128
                    qsl = slice(q0 + qoff, q0 + QB)
                    psS = pps.tile([128, 2 * QB], F32, name="psS", tag="psS")
                    nc.tensor.matmul(psS[:, qoff:QB], kR[0:64, ksl],
                                     qR[0:64, qsl], start=True, stop=True,
                                     tile_position=(0, 0))
                    nc.tensor.matmul(psS[:, QB + qoff:2 * QB], kR[64:128, ksl],
                                     qR[64:128, qsl], start=True, stop=True,
                                     tile_position=(64, 0))
                    A = pbA.tile([128, 2 * QB], BF16, name="A", tag="A")
                    nc.scalar.activation(A[:, qoff:2 * QB], psS[:, qoff:2 * QB],
                                         ACT.Exp, bias=lnisx[:, kt:kt + 1],
                                         scale=ISQ)
                    if v >= 0:
                        nc.vector.tensor_tensor(A[:, qoff:qoff + 128],
                                                A[:, qoff:qoff + 128],
                                                umask[:], OP.mult)
                        nc.vector.tensor_tensor(A[:, QB + qoff:QB + qoff + 128],
                                                A[:, QB + qoff:QB + qoff + 128],
                                                umask[:], OP.mult)
                    st, sp = kl == 0, kl == nkt - 1
                    nc.tensor.matmul(psA0[:, qoff:QB],
                                     vaug[:, kt * VW:kt * VW + 65],
                                     A[:, qoff:QB], start=st, stop=sp)
                    nc.tensor.matmul(psA1[:, qoff:QB],
                                     vaug[:, kt * VW + 65:kt * VW + 130],
                                     A[:, QB + qoff:2 * QB], start=st, stop=sp)
                # evacuate numerators + denominator row
                nc.vector.tensor_copy(outU[0:65, q0:q0 + QB], psA0[:])
                nc.vector.tensor_copy(outU[0:65, NT + q0:NT + q0 + QB], psA1[:])
                # denominator reciprocal on 32 lanes
                rsq = pbq.tile([32, 32], F32, name="rsq", tag="rsq")
                nc.sync.dma_start(rsq[0:16, :], outU[64:65, q0:q0 + QB])
                nc.sync.dma_start(rsq[16:32, :], outU[64:65, NT + q0:NT + q0 + QB])
                rrec = pbq.tile([32, 32], F32, name="rrec", tag="rrec")
                nc.vector.reciprocal(rrec[:], rsq[:])
                nc.sync.dma_start(rinv[2 * bi:2 * bi + 1, :], rrec[0:16, :])
                nc.sync.dma_start(rinv[2 * bi + 1:2 * bi + 2, :], rrec[16:32, :])
                # renormalize in place
                for h in range(2):
                    r = 2 * bi + h
                    brs = ppb.tile([64, QB], F32, name="brs", tag="bb")
                    nc.tensor.matmul(brs[:], sel16[:, r * 64:(r + 1) * 64],
                                     r32(rinv[0:16, :]),
                                     start=True, stop=True)
                    colU = slice(h * NT + q0, h * NT + q0 + QB)
                    nc.vector.tensor_tensor(outU[0:64, colU], outU[0:64, colU],
                                            brs[:], OP.mult)
                # partial absmax over this core's 128 dims
                par0 = pb.tile([64, QB], F32, name="par0", tag="par0", bufs=1)
                nc.gpsimd.partition_all_reduce(par0[:], outU[0:64, q0:q0 + QB],
                                               channels=64,
                                               reduce_op=bass_isa.ReduceOp.absmax)
                par1 = pb.tile([64, QB], F32, name="par1", tag="par1", bufs=1)
                nc.gpsimd.partition_all_reduce(par1[:],
                                               outU[0:64, NT + q0:NT + q0 + QB],
                                               channels=64,
                                               reduce_op=bass_isa.ReduceOp.absmax)
                nc.vector.tensor_tensor(par0[0:1, :], par0[0:1, :],
                                        par1[0:1, :], OP.max)
                # ship renormalized dims + partial-max rows into the a2a
                # payload, split by token-owner quarter
                j = bi % NQB
                for half in range(2):
                    owner = 2 * j + half
                    tsl = slice(q0 + half * OT, q0 + (half + 1) * OT)
                    for h in range(2):
                        rr = PR * owner + 64 * h
                        nc.sync.dma_start(
                            a2a_in[b][rr:rr + 64, :],
                            outU[0:64, h * NT + q0 + half * OT:
                                 h * NT + q0 + (half + 1) * OT])
                    nc.sync.dma_start(a2a_in[b][PR * owner + 128:PR * owner + 129, :],
                                      par0[0:1, half * OT:(half + 1) * OT])

            def a2a_chunk(c):
                nc.gpsimd.collective_compute(
                    "AllToAll", OP.bypass, replica_groups=RG,
                    ins=[a2a_in[c][:].opt()], outs=[a2a_out[c][:].opt()])

            def owner_chunk(c):
                """receive batch c: global scale, quantize, wo, output."""
                gsb = []
                pm = pbq.tile([8, OT], F32, name="pm", tag="pm")
                for i in range(8):
                    g = pbq.tile([128, OT], F32, name=f"g{i}", tag=f"g{i}")
                    nc.sync.dma_start(g[:], a2a_out[c][i * PR:i * PR + 128, :])
                    gsb.append(g)
                    nc.sync.dma_start(pm[i:i + 1, :],
                                      a2a_out[c][i * PR + 128:(i + 1) * PR, :])
                pmr = pbq.tile([8, OT], F32, name="pmr", tag="pmr")
                nc.gpsimd.partition_all_reduce(pmr[:], pm[:], channels=8,
                                               reduce_op=bass_isa.ReduceOp.max)
                gm2 = pbq.tile([1, OT], F32, name="gm2", tag="gm2")
                nc.vector.tensor_scalar(gm2[:], pmr[0:1, :], 1e-5, None, OP.add)
                rec = pbq.tile([1, OT], F32, name="rec", tag="rec")
                nc.vector.reciprocal(rec[:], gm2[:])
                so_row = pbq.tile([1, OT], F32R, name="so_row", tag="so_row")
                nc.vector.tensor_scalar(so_row[:], rec[:], 127.0, None, OP.mult)
                iso_row = pbq.tile([1, OT], F32R, name="iso_row", tag="iso_row")
                nc.vector.tensor_scalar(iso_row[:], gm2[:], 1.0 / 127.0, None,
                                        OP.mult)
                sob = ppb.tile([128, OT], F32, name="sob", tag="bb")
                nc.tensor.matmul(sob[:], ones1[:], so_row[:],
                                 start=True, stop=True)
                sos = pb.tile([128, OT], F32, name="sos", tag="sos", bufs=1)
                nc.scalar.copy(sos[:], sob[:])
                isob = ppb.tile([128, OT], F32, name="isob", tag="bb")
                nc.tensor.matmul(isob[:], ones1[:], iso_row[:],
                                 start=True, stop=True)
                isos = pb.tile([128, OT], F32, name="isos", tag="isos", bufs=1)
                nc.scalar.copy(isos[:], isob[:])
                xq = []
                for i in range(8):
                    yq = pb.tile([128, OT], F32, name="yq", tag="yq")
                    nc.vector.tensor_tensor(yq[:], gsb[i][0:128, :], sos[:],
                                            OP.mult)
                    xqi = pbq.tile([128, OT], BF16, name=f"xq{i}", tag=f"xq{i}")
                    nc.vector.tensor_scalar(xqi[:], yq[:], RC, RC,
                                            OP.add, OP.subtract)
                    xq.append(xqi)
                for oc in range(8):
                    pw = ppb.tile([128, OT], F32, name="pw", tag="bb")
                    for i in range(8):
                        nc.tensor.matmul(
                            pw[:],
                            w_sb[i][:, 3 * DPC + oc * 128:3 * DPC + (oc + 1) * 128],
                            xq[i][:], start=(i == 0), stop=(i == 7))
                    fin = pb.tile([128, OT], F32, name="fin", tag="fin")
                    nc.vector.tensor_tensor(fin[:], pw[:], isos[:], OP.mult)
                    nc.sync.dma_start(
                        out[oc * 128:(oc + 1) * 128, c * OT:(c + 1) * OT], fin[:])

            # Interleave: batch-1 projections fill the PE slack of batch-0's
            # exp-bound attention; the owner-side work for batch 0 runs in
            # the slack of blocks 5-7
            proj_pair(0)
            proj_pair(1)
            attention_block(0)
            attention_block(1)
            proj_pair(2)
            attention_block(2)
            proj_pair(3)
            attention_block(3)
            a2a_chunk(0)
            # wo weights: needed only by owner_chunk(0), ~100us from now;
            # loading them here keeps them out of Phase A's DMA window
            for i in range(8):
                nc.sync.dma_start(w_sb[i][:, 3 * DPC:],
                                  io["wall"][i * 128:(i + 1) * 128, 3 * DPC:])
            attention_block(4)
            attention_block(5)
            attention_block(6)
            owner_chunk(0)
            attention_block(7)
            a2a_chunk(1)
            owner_chunk(1)

    return nc


_CACHE = {}


def kernel(x, cos, sin, wq_w, wk_w, wv_w, wo_w):
    x = np.asarray(x, np.float32)
    cos = np.asarray(cos, np.float32)   # [T, 32]
    sin = np.asarray(sin, np.float32)
    xf = np.ascontiguousarray(x.reshape(NT, D))

    amax = np.abs(xf).max(-1) + 1e-5
    sx = (127.0 / amax).astype(np.float32)
    isx = (amax / 127.0).astype(np.float32)
    xq = np.clip(np.round(xf * sx[:, None]), -128.0, 127.0)
    xiT = np.ascontiguousarray(xq.T).astype(ml_dtypes.bfloat16)  # [D, NT]

    # RoPE tables (interleaved-pair convention) with isx folded in
    cm64 = np.repeat(cos.T, 2, axis=0)            # [64, T]
    sm64 = np.repeat(sin.T, 2, axis=0)
    cmap = np.tile(np.concatenate([cm64, cm64], axis=0), (1, B))
    smap = np.tile(np.concatenate([sm64, sm64], axis=0), (1, B))
    cmx = (cmap * isx[None, :]).astype(ml_dtypes.bfloat16)
    smx = (smap * isx[None, :]).astype(ml_dtypes.bfloat16)

    lnisx = np.ascontiguousarray(np.log(isx).reshape(NTT, 128).T).astype(np.float32)
    sxp = np.ascontiguousarray(sx.reshape(NTT, 128).T).astype(ml_dtypes.bfloat16)

    kk = np.arange(128)[:, None]
    jj = np.arange(128)[None, :]
    umask = (jj >= kk).astype(ml_dtypes.bfloat16)

    P = np.zeros((128, 128), np.float32)
    for j in range(64):
        P[2 * j, 2 * j + 1] = -1.0
        P[2 * j + 1, 2 * j] = 1.0
    pswapT = np.ascontiguousarray(P.T).astype(ml_dtypes.bfloat16)
    identb = np.eye(128, dtype=ml_dtypes.bfloat16)
    sel16 = np.zeros((16, 16 * 64), np.float32)
    for r in range(16):
        sel16[r, r * 64:(r + 1) * 64] = 1.0
    ones1 = np.ones((1, 128), np.float32)

    wq_e, wk_e, wv_e, wo_e = (_quant_w(np.asarray(w, np.float32))
                              for w in (wq_w, wk_w, wv_w, wo_w))
    woT = np.ascontiguousarray(wo_e.T)   # [D, D] full

    if "nc" not in _CACHE:
        nc0 = build_nc()
        nc0.finalize()
        _CACHE["nc"] = nc0
    nc = _CACHE["nc"]

    in_maps = []
    for c in range(N_CORES):
        hs = slice(c * DPC, (c + 1) * DPC)
        wall = np.concatenate(
            [np.ascontiguousarray(w[hs, :].T) for w in (wq_e, wk_e, wv_e)]
            + [woT], axis=1).astype(ml_dtypes.bfloat16)   # [D, 384+1024]
        in_maps.append({
            "xiT": xiT, "wall": wall, "cmx": cmx, "smx": smx,
            "lnisx": lnisx, "sxp": sxp, "umask": umask,
            "pswapT": pswapT, "identb": identb, "sel16": sel16, "ones1": ones1,
        })

    res = run_bass_kernel_spmd(nc, in_maps, core_ids=list(range(N_CORES)))
    outp = np.empty((NT, D), np.float32)
    for c in range(N_CORES):
        o = res.results[c]["out"]          # [D, 2*OT]
        outp[c * OT:(c + 1) * OT, :] = o[:, 0:OT].T
        outp[T + c * OT:T + (c + 1) * OT, :] = o[:, OT:2 * OT].T
    return outp.reshape(B, T, D)


# revision 49
# speedup vs baseline: 1.1699x; 1.0119x over previous
"""BitNet-style attention (B=2, T=2048, D=1024, 16 heads, RoPE, causal) on
8 TRN2 NeuronCores.

Head-parallel attention (2 heads/core) with a token-parallel wo:
  - host pre-quantizes x (int-valued bf16, transposed) and the ternary
    weights; per-token dequant scales (isx) are folded into the RoPE
    tables (q,k), the exp bias (A' = A*isx_k), and an augmented V column
    (1/isx_k = sx) that yields the softmax denominator for free.
  - scores: 2 heads row-packed on the PE (K=64 each); causal blocks get
    a restricted moving dim; the 128-wide diagonal is masked post-exp.
  - batch-1 projections are interleaved into batch-0's (exp-bound)
    attention stream through a shared PSUM pool set.
  - one AllToAll per batch reshards the renormalized f32 attention out
    by token owner (128 dims + a partial-absmax row per core); owners
    compute the int8 scale, quantize, and their 512-token slice of wo
    locally -- no AllReduce / AllGather on the critical path.
"""

import math
from contextlib import ExitStack

import ml_dtypes
import numpy as np

import concourse.bass as bass
import concourse.bacc as bacc_mod
import concourse.bass_isa as bass_isa
import concourse.mybir as mybir
import concourse.tile as tile
from concourse.bass_utils import run_bass_kernel_spmd

F32 = mybir.dt.float32
F32R = mybir.dt.float32r
BF16 = mybir.dt.bfloat16
OP = mybir.AluOpType
ACT = mybir.ActivationFunctionType

B, T, D = 2, 2048, 1024
NT = B * T              # 4096 tokens
NH, HD = 16, 64
N_CORES = 8
HPC = NH // N_CORES     # heads/core = 2
DPC = HPC * HD          # dims/core = 128
RC = 12582912.0         # 1.5*2^23 round-to-nearest-even constant

TB = 512                # token block (matmul N)
NTB = NT // TB          # 8
NTT = NT // 128         # 32 token tiles
QB = 512                # q block
NQB = T // QB           # 4 per batch
NKT = T // 128          # 16 k tiles per batch
VW = 130                # vaug group width: [v_h0(64) | sx | v_h1(64) | sx]
OT = T // N_CORES       # tokens owned per core per batch = 256
PR = 129                # a2a part rows: 128 dims + 1 partial-max row
ISQ = 1.0 / math.sqrt(HD)


def _quant_w(w):
    O, I = w.shape
    wg = w.reshape(O, I // 128, 128)
    ws = np.abs(wg).mean(-1, keepdims=True) + 1e-5
    wq = np.clip(np.round(wg / ws), -1.0, 1.0) * ws
    return wq.reshape(O, I).astype(np.float32)


def build_nc():
    nc = bacc_mod.Bacc(num_devices=N_CORES)
    io = {}

    def inp(name, shape, dt=F32):
        io[name] = nc.dram_tensor(name, shape, dt, kind="ExternalInput")

    inp("xiT", [D, NT], BF16)        # quantized x, transposed (int-valued)
    inp("wall", [D, 3 * DPC + D], BF16)  # [wq|wk|wv slices | full woT]
    inp("cmx", [128, NT], BF16)      # cos table * isx
    inp("smx", [128, NT], BF16)      # sin table * isx
    inp("lnisx", [128, NTT], F32)    # ln(isx) laid out [token%128, tile]
    inp("sxp", [128, NTT], BF16)     # sx laid out [token%128, tile]
    inp("umask", [128, 128], BF16)   # tri mask (1 if q>=k)
    inp("pswapT", [128, 128], BF16)  # RoPE pair swap
    inp("identb", [128, 128], BF16)
    inp("sel16", [16, 16 * 64], F32R)   # one-hot row selectors (renorm bcast)
    inp("ones1", [1, 128], F32R)
    out = nc.dram_tensor("out", [D, 2 * OT], F32, kind="ExternalOutput")

    r32 = lambda ap: ap.bitcast(F32R)
    RG = [list(range(N_CORES))]

    with nc.allow_low_precision(reason="bf16 matmul pipeline on int-exact activations"), \
         tile.TileContext(nc) as tc, ExitStack() as top:
        cpool = top.enter_context(tc.tile_pool(name="const", bufs=1))
        dpool = top.enter_context(tc.tile_pool(name="dram", bufs=1, space="DRAM"))

        # ---------------- persistent tiles ----------------
        w_sb = [cpool.tile([128, 3 * DPC + D], BF16, name=f"w{i}", tag=f"w{i}")
                for i in range(8)]
        xi_sb = [cpool.tile([128, NT], BF16, name=f"xi{i}", tag=f"xi{i}")
                 for i in range(8)]
        lnisx = cpool.tile([128, NTT], F32, name="lnisx", tag="lnisx")
        sxp = cpool.tile([128, NTT], BF16, name="sxp", tag="sxp")
        umask = cpool.tile([128, 128], BF16, name="umask", tag="umask")
        pswapT = cpool.tile([128, 128], BF16, name="pswapT", tag="pswapT")
        identb = cpool.tile([128, 128], BF16, name="identb", tag="identb")
        sel16 = cpool.tile([16, 16 * 64], F32R, name="sel16", tag="sel16")
        ones1 = cpool.tile([1, 128], F32R, name="ones1", tag="ones1")

        qR = cpool.tile([128, NT], BF16, name="qR", tag="qR")
        kR = cpool.tile([128, NT], BF16, name="kR", tag="kR")
        vaug = cpool.tile([128, NTT * VW], BF16, name="vaug", tag="vaug")
        # unnormalized attention out + denominator row (row 64);
        # head0 cols [0:NT), head1 cols [NT:2NT)
        outU = cpool.tile([65, 2 * NT], F32, name="outU", tag="outU")
        rinv = cpool.tile([16, QB], F32, name="rinv", tag="rinv")

        # DRAM scratch: AllToAll payload = 8 parts x (128 dim rows + 1
        # partial-max row) x 256 owned tokens, f32
        a2a_in = [dpool.tile([8 * PR, OT], F32, name=f"a2a_in{c}",
                             tag=f"a2a_in{c}") for c in range(B)]
        a2a_out = [dpool.tile([8 * PR, OT], F32, name=f"a2a_out{c}",
                              tag=f"a2a_out{c}") for c in range(B)]

        # input DMAs, all on the sync ring, in consumption order
        nc.sync.dma_start(identb[:], io["identb"][:])
        for nm, t in (("lnisx", lnisx), ("sxp", sxp), ("umask", umask),
                      ("pswapT", pswapT), ("sel16", sel16), ("ones1", ones1)):
            nc.sync.dma_start(t[:], io[nm][:])
        for i in range(8):
            nc.sync.dma_start(w_sb[i][:, 0:3 * DPC],
                              io["wall"][i * 128:(i + 1) * 128, 0:3 * DPC])
        # staged rows are contracted against one-hot selectors before every
        # row is written; zero-init so 0*garbage can't produce NaN
        nc.vector.memset(rinv[:], 0.0)
        # x in 256 KB slices, token-pair-major
        for tp in range(4):
            sl = slice(tp * 1024, (tp + 1) * 1024)
            for i in range(8):
                nc.sync.dma_start(xi_sb[i][:, sl],
                                  io["xiT"][i * 128:(i + 1) * 128, sl])
        # PE warm-up: ~13 us of dependency-free matmuls so the HAM clock
        # gate opens while the input DMAs stream in
        with tc.tile_pool(name="pwm", bufs=1, space="PSUM") as pwm:
            warm = pwm.tile([128, 128], F32, name="warm", tag="warm")
            for _ in range(120):
                nc.tensor.matmul(warm[:], identb[:], identb[:],
                                 start=True, stop=True)

        # prefill vaug sx columns (cols 64 and 129 of each 130-wide group)
        for kt in range(NTT):
            nc.vector.tensor_copy(vaug[:, kt * VW + 64:kt * VW + 65],
                                  sxp[:, kt:kt + 1])
            nc.vector.tensor_copy(vaug[:, kt * VW + 129:kt * VW + 130],
                                  sxp[:, kt:kt + 1])

        # One shared PSUM pool set for both phases so their emission can be
        # interleaved: psS 2x[128,1024] (4 banks) + psA0/psA1 (2) + bb
        # 2x[128,512] (2) = 8 banks
        with tc.tile_pool(name="pps", bufs=2, space="PSUM") as pps, \
             tc.tile_pool(name="ppa", bufs=1, space="PSUM") as ppa, \
             tc.tile_pool(name="ppb", bufs=2, space="PSUM") as ppb, \
             tc.tile_pool(name="pa", bufs=2) as pa, \
             tc.tile_pool(name="pb", bufs=2) as pb, \
             tc.tile_pool(name="pbA", bufs=3) as pbA, \
             tc.tile_pool(name="pbq", bufs=1) as pbq:

            def proj_pair(tp):
                """qkv + RoPE + V transpose for token blocks 2tp, 2tp+1."""
                tbs = (2 * tp, 2 * tp + 1)
                sls = [slice(tb * TB, (tb + 1) * TB) for tb in tbs]
                cmb = pa.tile([128, 2 * TB], BF16, name="cmb", tag="cmb")
                nc.sync.dma_start(cmb[:], io["cmx"][:, tp * 1024:(tp + 1) * 1024])
                smb = pa.tile([128, 2 * TB], BF16, name="smb", tag="smb")
                nc.sync.dma_start(smb[:], io["smx"][:, tp * 1024:(tp + 1) * 1024])
                blk = {}
                for pi, pname in enumerate(("q", "k", "v")):
                    pp2 = pps.tile([128, 2 * TB], F32, name="pp2", tag="psS")
                    for i in range(8):
                        for u in range(2):
                            nc.tensor.matmul(pp2[:, u * TB:(u + 1) * TB],
                                             w_sb[i][:, pi * 128:(pi + 1) * 128],
                                             xi_sb[i][:, sls[u]],
                                             start=(i == 0), stop=(i == 7))
                    for u in range(2):
                        t = pa.tile([128, TB], BF16, name=f"t_{pname}{u}",
                                    tag=f"t_{pname}{u}")
                        if pname == "v":
                            nc.vector.tensor_copy(t[:], pp2[:, u * TB:(u + 1) * TB])
                        else:
                            nc.scalar.copy(t[:], pp2[:, u * TB:(u + 1) * TB])
                        blk[(pname, u)] = t
                for u in range(2):
                    sl = sls[u]
                    csl = slice(u * TB, (u + 1) * TB)
                    for pname, dstR in (("q", qR), ("k", kR)):
                        src = blk[(pname, u)]
                        swp = ppb.tile([128, TB], F32, name="swp", tag="bb")
                        nc.tensor.matmul(swp[:], pswapT[:], src[:],
                                         start=True, stop=True)
                        t1 = pa.tile([128, TB], BF16, name="t1", tag="t1")
                        nc.vector.tensor_tensor(t1[:], src[:], cmb[:, csl], OP.mult)
                        t2 = pa.tile([128, TB], BF16, name="t2", tag="t2")
                        nc.vector.tensor_tensor(t2[:], swp[:], smb[:, csl], OP.mult)
                        nc.vector.tensor_tensor(dstR[:, sl], t1[:], t2[:], OP.add)
                    for j in range(TB // 128):
                        kt = tbs[u] * 4 + j
                        vtp = ppb.tile([128, 128], BF16, name="vtp", tag="bb")
                        nc.tensor.transpose(vtp[:],
                                            blk[("v", u)][:, j * 128:(j + 1) * 128],
                                            identb[:])
                        nc.vector.tensor_copy(vaug[:, kt * VW:kt * VW + 64],
                                              vtp[:, 0:64])
                        nc.vector.tensor_copy(vaug[:, kt * VW + 65:kt * VW + 129],
                                              vtp[:, 64:128])

            def attention_block(bi):
                b, qb = divmod(bi, NQB)
                q0 = b * T + qb * QB
                nkt = 4 * qb + 4
                psA0 = ppa.tile([65, QB], F32, name="psA0", tag="psA0")
                psA1 = ppa.tile([65, QB], F32, name="psA1", tag="psA1")
                for kl in range(nkt):
                    kt = b * NKT + kl
                    ksl = slice(kt * 128, (kt + 1) * 128)
                    v = kl - 4 * qb
                    qoff = max(v, 0) * 128
                    qsl = slice(q0 + qoff, q0 + QB)
                    psS = pps.tile([128, 2 * QB], F32, name="psS", tag="psS")
                    nc.tensor.matmul(psS[:, qoff:QB], kR[0:64, ksl],
                                     qR[0:64, qsl], start=True, stop=True,
                                     tile_position=(0, 0))
                    nc.tensor.matmul(psS[:, QB + qoff:2 * QB], kR[64:128, ksl],
                                     qR[64:128, qsl], start=True, stop=True,
                                     tile_position=(64, 0))
                    A = pbA.tile([128, 2 * QB], BF16, name="A", tag="A")
                    nc.scalar.activation(A[:, qoff:2 * QB], psS[:, qoff:2 * QB],
                                         ACT.Exp, bias=lnisx[:, kt:kt + 1],
                                         scale=ISQ)
                    if v >= 0:
                        nc.vector.tensor_tensor(A[:, qoff:qoff + 128],
                                                A[:, qoff:qoff + 128],
                                                umask[:], OP.mult)
                        nc.vector.tensor_tensor(A[:, QB + qoff:QB + qoff + 128],
                                                A[:, QB + qoff:QB + qoff + 128],
                                                umask[:], OP.mult)
                    st, sp = kl == 0, kl == nkt - 1
                    nc.tensor.matmul(psA0[:, qoff:QB],
                                     vaug[:, kt * VW:kt * VW + 65],
                                     A[:, qoff:QB], start=st, stop=sp)
                    nc.tensor.matmul(psA1[:, qoff:QB],
                                     vaug[:, kt * VW + 65:kt * VW + 130],
                                     A[:, QB + qoff:2 * QB], start=st, stop=sp)
                # evacuate numerators + denominator row
                nc.vector.tensor_copy(outU[0:65, q0:q0 + QB], psA0[:])
                nc.vector.tensor_copy(outU[0:65, NT + q0:NT + q0 + QB], psA1[:])
                # denominator reciprocal on 32 lanes
                rsq = pbq.tile([32, 32], F32, name="rsq", tag="rsq")
                nc.sync.dma_start(rsq[0:16, :], outU[64:65, q0:q0 + QB])
                nc.sync.dma_start(rsq[16:32, :], outU[64:65, NT + q0:NT + q0 + QB])
                rrec = pbq.tile([32, 32], F32, name="rrec", tag="rrec")
                nc.vector.reciprocal(rrec[:], rsq[:])
                nc.sync.dma_start(rinv[2 * bi:2 * bi + 1, :], rrec[0:16, :])
                nc.sync.dma_start(rinv[2 * bi + 1:2 * bi + 2, :], rrec[16:32, :])
                # renormalize in place
                for h in range(2):
                    r = 2 * bi + h
                    brs = ppb.tile([64, QB], F32, name="brs", tag="bb")
                    nc.tensor.matmul(brs[:], sel16[:, r * 64:(r + 1) * 64],
                                     r32(rinv[0:16, :]),
                                     start=True, stop=True)
                    colU = slice(h * NT + q0, h * NT + q0 + QB)
                    nc.vector.tensor_tensor(outU[0:64, colU], outU[0:64, colU],
                                            brs[:], OP.mult)
                # partial absmax over this core's 128 dims
                par0 = pb.tile([64, QB], F32, name="par0", tag="par0", bufs=1)
                nc.gpsimd.partition_all_reduce(par0[:], outU[0:64, q0:q0 + QB],
                                               channels=64,
                                               reduce_op=bass_isa.ReduceOp.absmax)
                par1 = pb.tile([64, QB], F32, name="par1", tag="par1", bufs=1)
                nc.gpsimd.partition_all_reduce(par1[:],
                                               outU[0:64, NT + q0:NT + q0 + QB],
                                               channels=64,
                                               reduce_op=bass_isa.ReduceOp.absmax)
                nc.vector.tensor_tensor(par0[0:1, :], par0[0:1, :],
                                        par1[0:1, :], OP.max)
                # ship renormalized dims + partial-max rows into the a2a
                # payload, split by token-owner quarter
                j = bi % NQB
                for half in range(2):
                    owner = 2 * j + half
                    tsl = slice(q0 + half * OT, q0 + (half + 1) * OT)
                    for h in range(2):
                        rr = PR * owner + 64 * h
                        nc.sync.dma_start(
                            a2a_in[b][rr:rr + 64, :],
                            outU[0:64, h * NT + q0 + half * OT:
                                 h * NT + q0 + (half + 1) * OT])
                    nc.sync.dma_start(a2a_in[b][PR * owner + 128:PR * owner + 129, :],
                                      par0[0:1, half * OT:(half + 1) * OT])

            def a2a_chunk(c):
                nc.gpsimd.collective_compute(
                    "AllToAll", OP.bypass, replica_groups=RG,
                    ins=[a2a_in[c][:].opt()], outs=[a2a_out[c][:].opt()])

            def owner_chunk(c):
                """receive batch c: global scale, quantize, wo, output."""
                gsb = []
                pm = pbq.tile([8, OT], F32, name="pm", tag="pm")
                for i in range(8):
                    g = pbq.tile([128, OT], F32, name=f"g{i}", tag=f"g{i}")
                    nc.sync.dma_start(g[:], a2a_out[c][i * PR:i * PR + 128, :])
                    gsb.append(g)
                    nc.sync.dma_start(pm[i:i + 1, :],
                                      a2a_out[c][i * PR + 128:(i + 1) * PR, :])
                pmr = pbq.tile([8, OT], F32, name="pmr", tag="pmr")
                nc.gpsimd.partition_all_reduce(pmr[:], pm[:], channels=8,
                                               reduce_op=bass_isa.ReduceOp.max)
                gm2 = pbq.tile([1, OT], F32, name="gm2", tag="gm2")
                nc.vector.tensor_scalar(gm2[:], pmr[0:1, :], 1e-5, None, OP.add)
                rec = pbq.tile([1, OT], F32, name="rec", tag="rec")
                nc.vector.reciprocal(rec[:], gm2[:])
                so_row = pbq.tile([1, OT], F32R, name="so_row", tag="so_row")
                nc.vector.tensor_scalar(so_row[:], rec[:], 127.0, None, OP.mult)
                iso_row = pbq.tile([1, OT], F32R, name="iso_row", tag="iso_row")
                nc.vector.tensor_scalar(iso_row[:], gm2[:], 1.0 / 127.0, None,
                                        OP.mult)
                sob = ppb.tile([128, OT], F32, name="sob", tag="bb")
                nc.tensor.matmul(sob[:], ones1[:], so_row[:],
                                 start=True, stop=True)
                sos = pb.tile([128, OT], F32, name="sos", tag="sos", bufs=1)
                nc.scalar.copy(sos[:], sob[:])
                isob = ppb.tile([128, OT], F32, name="isob", tag="bb")
                nc.tensor.matmul(isob[:], ones1[:], iso_row[:],
                                 start=True, stop=True)
                isos = pb.tile([128, OT], F32, name="isos", tag="isos", bufs=1)
                nc.scalar.copy(isos[:], isob[:])
                xq = []
                for i in range(8):
                    yq = pb.tile([128, OT], F32, name="yq", tag="yq")
                    nc.vector.tensor_tensor(yq[:], gsb[i][0:128, :], sos[:],
                                            OP.mult)
                    xqi = pbq.tile([128, OT], BF16, name=f"xq{i}", tag=f"xq{i}")
                    nc.vector.tensor_scalar(xqi[:], yq[:], RC, RC,
                                            OP.add, OP.subtract)
                    xq.append(xqi)
                for oc in range(8):
                    pw = ppb.tile([128, OT], F32, name="pw", tag="bb")
                    for i in range(8):
                        nc.tensor.matmul(
                            pw[:],
                            w_sb[i][:, 3 * DPC + oc * 128:3 * DPC + (oc + 1) * 128],
                            xq[i][:], start=(i == 0), stop=(i == 7))
                    fin = pb.tile([128, OT], F32, name="fin", tag="fin")
                    nc.vector.tensor_tensor(fin[:], pw[:], isos[:], OP.mult)
                    nc.sync.dma_start(
                        out[oc * 128:(oc + 1) * 128, c * OT:(c + 1) * OT], fin[:])

            # Interleave: batch-1 projections fill the PE slack of batch-0's
            # exp-bound attention; the owner-side work for batch 0 runs in
            # the slack of blocks 5-7
            proj_pair(0)
            proj_pair(1)
            attention_block(0)
            attention_block(1)
            proj_pair(2)
            attention_block(2)
            proj_pair(3)
            attention_block(3)
            a2a_chunk(0)
            # wo weights: needed only by owner_chunk(0), ~100us from now;
            # loading them here keeps them out of Phase A's DMA window
            for i in range(8):
                nc.sync.dma_start(w_sb[i][:, 3 * DPC:],
                                  io["wall"][i * 128:(i + 1) * 128, 3 * DPC:])
            attention_block(4)
            attention_block(5)
            attention_block(6)
            attention_block(7)
            a2a_chunk(1)
            # owner-0's scale/quant/wo fills the PE while the batch-1
            # AllToAll is in flight
            owner_chunk(0)
            owner_chunk(1)

    return nc


_CACHE = {}


def kernel(x, cos, sin, wq_w, wk_w, wv_w, wo_w):
    x = np.asarray(x, np.float32)
    cos = np.asarray(cos, np.float32)   # [T, 32]
    sin = np.asarray(sin, np.float32)
    xf = np.ascontiguousarray(x.reshape(NT, D))

    amax = np.abs(xf).max(-1) + 1e-5
    sx = (127.0 / amax).astype(np.float32)
    isx = (amax / 127.0).astype(np.float32)
    xq = np.clip(np.round(xf * sx[:, None]), -128.0, 127.0)
    xiT = np.ascontiguousarray(xq.T).astype(ml_dtypes.bfloat16)  # [D, NT]

    # RoPE tables (interleaved-pair convention) with isx folded in
    cm64 = np.repeat(cos.T, 2, axis=0)            # [64, T]
    sm64 = np.repeat(sin.T, 2, axis=0)
    cmap = np.tile(np.concatenate([cm64, cm64], axis=0), (1, B))
    smap = np.tile(np.concatenate([sm64, sm64], axis=0), (1, B))
    cmx = (cmap * isx[None, :]).astype(ml_dtypes.bfloat16)
    smx = (smap * isx[None, :]).astype(ml_dtypes.bfloat16)

    lnisx = np.ascontiguousarray(np.log(isx).reshape(NTT, 128).T).astype(np.float32)
    sxp = np.ascontiguousarray(sx.reshape(NTT, 128).T).astype(ml_dtypes.bfloat16)

    kk = np.arange(128)[:, None]
    jj = np.arange(128)[None, :]
    umask = (jj >= kk).astype(ml_dtypes.bfloat16)

    P = np.zeros((128, 128), np.float32)
    for j in range(64):
        P[2 * j, 2 * j + 1] = -1.0
        P[2 * j + 1, 2 * j] = 1.0
    pswapT = np.ascontiguousarray(P.T).astype(ml_dtypes.bfloat16)
    identb = np.eye(128, dtype=ml_dtypes.bfloat16)
    sel16 = np.zeros((16, 16 * 64), np.float32)
    for r in range(16):
        sel16[r, r * 64:(r + 1) * 64] = 1.0
    ones1 = np.ones((1, 128), np.float32)

    wq_e, wk_e, wv_e, wo_e = (_quant_w(np.asarray(w, np.float32))
                              for w in (wq_w, wk_w, wv_w, wo_w))
    woT = np.ascontiguousarray(wo_e.T)   # [D, D] full

    if "nc" not in _CACHE:
        nc0 = build_nc()
        nc0.finalize()
        _CACHE["nc"] = nc0
    nc = _CACHE["nc"]

    in_maps = []
    for c in range(N_CORES):
        hs = slice(c * DPC, (c + 1) * DPC)
        wall = np.concatenate(
            [np.ascontiguousarray(w[hs, :].T) for w in (wq_e, wk_e, wv_e)]
            + [woT], axis=1).astype(ml_dtypes.bfloat16)   # [D, 384+1024]
        in_maps.append({
            "xiT": xiT, "wall": wall, "cmx": cmx, "smx": smx,
            "lnisx": lnisx, "sxp": sxp, "umask": umask,
            "pswapT": pswapT, "identb": identb, "sel16": sel16, "ones1": ones1,
        })

    res = run_bass_kernel_spmd(nc, in_maps, core_ids=list(range(N_CORES)))
    outp = np.empty((NT, D), np.float32)
    for c in range(N_CORES):
        o = res.results[c]["out"]          # [D, 2*OT]
        outp[c * OT:(c + 1) * OT, :] = o[:, 0:OT].T
        outp[T + c * OT:T + (c + 1) * OT, :] = o[:, OT:2 * OT].T
    return outp.reshape(B, T, D)
